# revision 1
# baseline (speedup 1.0000x reference)
"""AttentionPooling Trainium2 kernel (8 NeuronCores, SPMD).

Reference computation:
    scores = tanh(x @ W1 + b1) @ W2          # [N, 4]
    w      = segment_softmax(scores, batch)  # per-graph softmax over nodes
    out[g] = mean_h( sum_{n in g} w[n,h] * x[n] )   # [G, 256]

Sharding: 64 graphs per core (512 graphs / 8 cores), LPT-bin-packed into
octs of 8 graphs so the largest oct is minimal; each oct's nodes are padded
to a fixed number of 128-node tiles (T) so every core runs the identical
instruction stream.  Weights are replicated; per-graph outputs are disjoint,
so the host concatenates the 8 core outputs and undoes the LPT permutation.

On-core algorithm (single pass over x, one 1024-node chunk at a time):
  - two DMA streams per chunk from one DRAM blob: the packed node-major row
    [x bf16 | 1 | bloc] on the SP-HWDGE path (two half-row transfers for a
    finer wire interleave), and x^T in e4m3 on the SWDGE path.  The x^T
    ring is deep (runs ~6 chunks ahead) and the xam ring is shallow, so
    the score-critical fp8 bytes win the early wire slots.  A PE warm-up
    burst and a dummy activation at t~0 pay the p-state ramp and the
    ~1.3us activation-table load before the first real data arrives.
  - z^T = W1^T @ x^T on TensorE as fp8 DoubleRow matmuls (K=256 folded into
    one pass); W1 is sent as an e4m3 hi+lo pair so its quantization error
    cancels (W1hi + W1lo ~ bf16-accurate), x^T is single e4m3
  - tanh(+per-partition bias) on ScalarE in two 1024-wide calls per chunk
    (one per h_out half) -> t^T bf16; ScalarE is the pacing engine, so exp
    is batched over two chunks to cut per-call overhead
  - s (node-major, 4 heads) = t^T-stationary matmuls, col-tiled into four
    M=32 tile_position groups; e = exp(s) once per 2 chunks.  NOTE: the
    matmul emission must stay j-outer (each region's start/stop pair close
    together) — a ko-outer order that interleaves 32 open accumulation
    groups breaks PSUM results on real hardware.
  - oct one-hot mask on DVE from the bloc column via a broadcast-AP
    iota-compare; sel[128, 32/tile] = e (broadcast over 8 oct slots) * mask
  - pooled[(oct%4)*32 + slot*4 + head, 0:256] += sel^T @ [x | 1] in bf16,
    accumulated across the whole shard in 2 persistent PSUM banks;
    column 256 (the ones column) gives the softmax denominator
  - epilogue per bank: clamp+divide by denominator, average heads with a
    constant matmul back into the same PSUM bank, copy out.  Bank A covers
    octs 0-3 and finishes mid-kernel, so its epilogue compute is emitted
    right after its last accumulation and hides under the main loop; the
    tiny out-DMAs run at the end (a mid-loop SP dma_start would stall the
    in-order SP queue, which prefetches chunks ahead).
"""

import contextlib as _ctxlib
import numpy as np
import ml_dtypes


def _nullctx():
    return _ctxlib.nullcontext()

BF16 = ml_dtypes.bfloat16
F8 = ml_dtypes.float8_e4m3  # IEEE e4m3 (max 240) == TRN FP8_EXP4

N_CORES = 8
H = 256
HEADS = 4
GRP = 8  # graphs per oct group
SELW = GRP * HEADS  # 32 selector columns per node
ROW = H + 2  # packed row: x(256) | ones(1) | bloc(1)
BLOC = H + 1  # bloc column index

XAMB = 8 * ROW * 2  # xam bytes per partition per chunk (4128)
XTB = 2 * 1024  # x^T fp8 bytes per partition per chunk (2048)
UB = XAMB + XTB  # merged stream bytes per partition per chunk (6176)
# consts blob byte offsets: W1hi | W1lo | W2 | b1 | shs | iota
C_HI, C_LO, C_W2, C_B1, C_SHS, C_IOT, C_END = 0, 512, 1024, 1040, 1048, 1112, 1176

_NC_CACHE = {}
LAST_RESULT = None


def _build_nc(T: int, n_grps: int, repeats: int = 1):
    """Build the SPMD Bass program. T = 128-node tiles per oct group."""
    import concourse.bacc as bacc
    import concourse.mybir as mybir
    from concourse.tile import TileContext

    fp32 = mybir.dt.float32
    bf16 = mybir.dt.bfloat16
    f8 = mybir.dt.float8e4
    u8 = mybir.dt.uint8
    AF = mybir.ActivationFunctionType
    DR = mybir.MatmulPerfMode.DoubleRow

    n_tiles = n_grps * T
    assert n_tiles % 8 == 0
    n_chunks = n_tiles // 8  # 1024-node chunks
    assert n_grps == 8, "psum layout assumes 8 octs (64 graphs) per core"
    ch_a_done = (4 * T - 1) // 8  # chunk in which poolA's last oct closes

    nc = bacc.Bacc(trn_type="TRN2")

    u = nc.dram_tensor("u", [n_chunks, 128, UB], u8, kind="ExternalInput")
    cdram = nc.dram_tensor("c", [128, C_END], u8, kind="ExternalInput")
    out = nc.dram_tensor("out", [64, H], fp32, kind="ExternalOutput")

    with TileContext(nc, pool_alloc_mode="queue") as tc:
        with (
            tc.tile_pool(name="consts", bufs=1) as cpool,
            tc.tile_pool(name="acc", bufs=1, space="PSUM") as acc_pool,
        ):
            cub = cpool.tile([128, C_END], u8)
            nc.sync.dma_start(cub[:], cdram.ap())
            # typed views into the consts blob
            w1hi = cub[:, C_HI:C_LO].bitcast(f8).rearrange("p (k m) -> p k m", k=2)
            w1lo = cub[:, C_LO:C_W2].bitcast(f8).rearrange("p (k m) -> p k m", k=2)
            w2v = cub[:, C_W2:C_B1].bitcast(bf16)  # [128, 8]
            b1v = cub[:, C_B1:C_SHS].bitcast(fp32)  # [128, 2]
            shsv = cub[:, C_SHS:C_IOT].bitcast(bf16)  # [128, 32]
            iotv = cub[:, C_IOT:C_END].bitcast(bf16)  # [128, 32]

            # persistent accumulators: rows = (oct%4)*32 + jj*4 + h, col 256 = seg_e
            poolA = acc_pool.tile([128, H + 1], fp32)
            poolB = acc_pool.tile([128, H + 1], fp32)

            def epilogue(ps, idx, wp, on_act):
                """Normalize by seg_e, head-mean back into ps[0:32].

                Returns the SBUF result tile; the caller issues the out-DMA.
                """
                seg = wp.tile([128, 1], fp32, name=f"seg{idx}", tag=f"seg{idx}")
                nc.vector.tensor_scalar(
                    seg[:], ps[:, H : H + 1], 1e-30, None, mybir.AluOpType.max
                )
                rec = wp.tile([128, 1], fp32, name=f"rec{idx}", tag=f"rec{idx}")
                nc.vector.reciprocal(rec[:], seg[:])
                norm = wp.tile([128, H], bf16, name=f"norm{idx}", tag=f"norm{idx}")
                nc.vector.tensor_scalar(
                    norm[:], ps[:, 0:H], rec[:], None, mybir.AluOpType.mult
                )
                # head-mean matmul reuses the accumulator's own PSUM bank
                nc.tensor.matmul(ps[0:32, 0:H], shsv, norm[:], start=True, stop=True)
                osb = wp.tile([32, H], fp32, name=f"osb{idx}", tag=f"osb{idx}")
                if on_act:
                    nc.scalar.copy(osb[:], ps[0:32, 0:H])
                else:
                    nc.vector.tensor_scalar(
                        osb[:], ps[0:32, 0:H], 0.0, None, mybir.AluOpType.add
                    )
                return osb

            # PE warm-up: zero matmuls keep the PE busy from t~0 so the
            # p-state ramp (2x slower below 3us) is over before chunk 0, and
            # a dummy activation on the same tile pulls the ~1.3us
            # LoadActFuncSet (tanh/exp table set) off the ramp
            wrm = cpool.tile([128, 257], bf16)
            dummy = cpool.tile([1, 8], bf16)
            with tc.high_priority():
                nc.vector.memset(wrm[:], 0.0)
                nc.scalar.activation(dummy[:], wrm[0:1, 0:8], AF.Tanh)
                for _w in range(14):
                    nc.tensor.matmul(
                        poolA[:], wrm[:, 0:128], wrm[:], start=True, stop=True
                    )

            with (
                # Two independent DMA rings.  The deep x^T ring lets the
                # fp8 score stream run ~8 chunks ahead on the wire (it
                # gates the ACT-critical tanh chain); the shallow xam ring
                # is throttled by its pool-matmul readers, so the bulky
                # node-major bytes naturally yield the early wire slots.
                tc.tile_pool(name="xt8", bufs=6) as xpool,
                tc.tile_pool(name="xam", bufs=4) as apool,
                tc.tile_pool(name="work", bufs=5) as wpool,
                tc.tile_pool(name="zt", bufs=1, space="PSUM") as zpool,
                tc.tile_pool(name="sps", bufs=2, space="PSUM") as spool,
            ):
              SB = 2  # chunks per exp superblock

              for _rep in range(repeats):
                s_ps = None
                ub_sb = [None] * SB
                osb_a = None
                for ch in range(n_chunks):
                    xt = xpool.tile([128, XTB], u8, name="xt")
                    nc.gpsimd.dma_start(xt[:], u.ap()[ch, :, XAMB:UB])
                    ub = apool.tile([128, XAMB], u8, name="am")
                    ub_sb[ch % SB] = ub
                    # two half-row DMAs: finer wire interleave, so the fp8
                    # x^T pieces get slots sooner during the ramp
                    nc.sync.dma_start(
                        ub[:, 0 : XAMB // 2], u.ap()[ch, :, 0 : XAMB // 2]
                    )
                    nc.sync.dma_start(
                        ub[:, XAMB // 2 : XAMB], u.ap()[ch, :, XAMB // 2 : XAMB]
                    )
                    xamv = ub[:].bitcast(bf16)  # [128, 2064]
                    xt8v = xt[:].bitcast(f8).rearrange(
                        "p (k n) -> p k n", k=2
                    )  # [128, 2, 1024]

                    # z^T = (W1hi + W1lo)^T @ x^T, fp8 DoubleRow (K=256/pass)
                    tt = wpool.tile([128, 2048], bf16, name="tt", tag="tt")
                    for ko in range(2):
                        zt = zpool.tile(
                            [128, 1024], fp32, name=f"zt{ko}", tag=f"zt{ko}"
                        )
                        for s2 in range(2):
                            # high priority: these gate the ACT-critical tanh
                            # chain and must not queue behind the pool backlog
                            with tc.high_priority():
                                nc.tensor.matmul(
                                    zt[:, s2 * 512 : (s2 + 1) * 512],
                                    w1hi[:, :, ko * 128 : ko * 128 + 128],
                                    xt8v[:, :, s2 * 512 : (s2 + 1) * 512],
                                    start=True,
                                    stop=False,
                                    perf_mode=DR,
                                )
                                nc.tensor.matmul(
                                    zt[:, s2 * 512 : (s2 + 1) * 512],
                                    w1lo[:, :, ko * 128 : ko * 128 + 128],
                                    xt8v[:, :, s2 * 512 : (s2 + 1) * 512],
                                    start=False,
                                    stop=True,
                                    perf_mode=DR,
                                )
                        # tanh(+bias): per-partition bias, one 1024-wide call
                        nc.scalar.activation(
                            tt[:, ko * 1024 : (ko + 1) * 1024],
                            zt[:],
                            AF.Tanh,
                            bias=b1v[:, ko : ko + 1],
                        )

                    # s (node-major): [128, (SB chunks) * 8 tiles * 4 heads].
                    # Each 128-node block is split into four M=32 col-group
                    # matmuls (tile_position) so the four ldweights stream
                    # through parallel XBUSes instead of one.  Keep j-outer:
                    # interleaving 32 open PSUM accumulation groups (ko-outer)
                    # corrupts results on real hardware.
                    cpar = ch % SB
                    if cpar == 0:
                        s_ps = spool.tile(
                            [128, SB * 8 * HEADS], fp32, name="s_ps", tag="s_ps"
                        )
                    for j in range(8):
                        for ko in range(2):
                            for cg in range(4):
                              with tc.high_priority():
                                nc.tensor.matmul(
                                    s_ps[
                                        cg * 32 : (cg + 1) * 32,
                                        cpar * 32 + j * HEADS : cpar * 32 + (j + 1) * HEADS,
                                    ],
                                    tt[
                                        :,
                                        ko * 1024 + j * 128 + cg * 32 : ko * 1024
                                        + j * 128
                                        + cg * 32
                                        + 32,
                                    ],
                                    w2v[:, ko * HEADS : (ko + 1) * HEADS],
                                    start=(ko == 0),
                                    stop=(ko == 1),
                                    tile_position=(0, cg * 32),
                                )
                    # e = exp(s), once per superblock (SB chunks)
                    last_in_sb = cpar == SB - 1 or ch == n_chunks - 1
                    if last_in_sb:
                        ew = (cpar + 1) * 32
                        e_sb = wpool.tile([128, SB * 8 * HEADS], bf16, name="e_sb")
                        # nudged later in the ACT queue: the s_ps->exp sem
                        # latency then hides under the next chunk's tanh.
                        # Near the end run it at natural priority instead, so
                        # the final superblocks' pool matmuls don't bunch up
                        # after the last tanh.
                        if ch >= n_chunks - 8:
                            nc.scalar.activation(e_sb[:, 0:ew], s_ps[:, 0:ew], AF.Exp)
                        else:
                            with tc.high_priority(offset=-45):
                                nc.scalar.activation(
                                    e_sb[:, 0:ew], s_ps[:, 0:ew], AF.Exp
                                )
                        for cc in range(cpar + 1):
                            cx = ch - cpar + cc
                            _chunk_tail(
                                nc, mybir, tc, wpool, ub_sb[cc], e_sb, cc, cx,
                                T, poolA, poolB, iotv,
                                last=(cx >= n_chunks - 3),
                            )
                            if cx == ch_a_done and _rep == repeats - 1:
                                # deprioritized: fills engine gaps, must not
                                # displace the steady-state pipeline
                                with tc.high_priority(offset=-100000):
                                    osb_a = epilogue(poolA, 0, wpool, on_act=False)

                # bank B epilogue (and bank A's if it didn't fire mid-loop)
                if _rep == repeats - 1:
                    if osb_a is None:
                        osb_a = epilogue(poolA, 0, wpool, on_act=False)
                    nc.sync.dma_start(out.ap()[0:32, :], osb_a[:])
                    with tc.high_priority():
                        osb_b = epilogue(poolB, 1, wpool, on_act=True)
                    nc.sync.dma_start(out.ap()[32:64, :], osb_b[:])

    nc.finalize()
    return nc


def _chunk_tail(
    nc, mybir, tc, wpool, ub, e_sb, cc, cx, T, poolA, poolB, iotv, last=False
):
    """Mask, selector, and pool matmuls for chunk cx (parity slot cc).

    For the final chunk the selector multiply is split into j-halves so the
    last pool matmuls (the end-of-program critical chain) start earlier.
    """
    bf16 = mybir.dt.bfloat16
    xamv = ub[:].bitcast(bf16)
    # oct one-hot masks: (bloc == iota), one broadcast op.  High priority:
    # no exp dependency, so it can run as soon as the xam bytes land instead
    # of queueing behind the superblock tail.
    mk = wpool.tile([128, 8 * SELW], bf16, name="mk")
    bloc_b = (
        xamv.rearrange("p (j c) -> p j c", j=8)[:, :, BLOC : BLOC + 1]
        .broadcast_to((128, 8, SELW))
    )
    iot_b = iotv.rearrange("p (o c) -> p o c", o=1).broadcast_to((128, 8, SELW))
    ctx = tc.high_priority() if last else _nullctx()
    with ctx:
        nc.vector.tensor_tensor(
            mk[:].rearrange("p (j c) -> p j c", j=8),
            bloc_b,
            iot_b,
            mybir.AluOpType.is_equal,
        )
    # selector = e * mask (e broadcast over the 8 oct slots)
    sel = wpool.tile([128, 8 * SELW], bf16, name="sel")
    for h0, h1 in ([(0, 4), (4, 8)] if last else [(0, 8)]):
        e_b = (
            e_sb[:, cc * SELW + h0 * HEADS : cc * SELW + h1 * HEADS]
            .rearrange("p (j o h) -> p j o h", j=h1 - h0, o=1)
            .broadcast_to((128, h1 - h0, GRP, HEADS))
        )
        nc.vector.tensor_tensor(
            sel[:, h0 * SELW : h1 * SELW].rearrange(
                "p (j o h) -> p j o h", j=h1 - h0, o=GRP
            ),
            e_b,
            mk[:, h0 * SELW : h1 * SELW].rearrange(
                "p (j o h) -> p j o h", j=h1 - h0, o=GRP
            ),
            mybir.AluOpType.mult,
        )
    # pooled[(o%4)*32 : +32, :] += sel_j^T @ [x_j | 1]
    for j in range(8):
        t_glob = cx * 8 + j
        o = t_glob // T
        tau = t_glob % T
        ps = poolA if (o % 8) < 4 else poolB
        r0 = (o % 4) * 32
        nc.tensor.matmul(
            ps[r0 : r0 + 32, :],
            sel[:, j * SELW : (j + 1) * SELW],
            xamv[:, j * ROW : j * ROW + H + 1],
            start=(tau == 0),
            stop=(tau == T - 1),
            tile_position=(0, r0),
        )


def _lpt_octs(counts, n_octs):
    """LPT-pack graphs into octs of GRP graphs, minimizing the max oct size."""
    import heapq

    order = np.argsort(-counts)
    heap = [(0, i, []) for i in range(n_octs)]
    heapq.heapify(heap)
    for g in order:
        popped = []
        while True:
            sz, i, lst = heapq.heappop(heap)
            if len(lst) < GRP:
                break
            popped.append((sz, i, lst))
        heapq.heappush(heap, (sz + int(counts[g]), i, lst + [int(g)]))
        for p in popped:
            heapq.heappush(heap, p)
    octs = [None] * n_octs
    for sz, i, lst in heap:
        octs[i] = lst
    return octs


def _host_prep(x, batch, W1, b1, W2, G):
    """Shard + pad inputs; build all per-core DRAM arrays."""
    gpc = G // N_CORES  # graphs per core
    n_grps = gpc // GRP  # oct groups per core
    counts = np.bincount(batch, minlength=G)
    octs = _lpt_octs(counts, G // GRP)  # balanced graph -> oct assignment
    oct_sums = np.array([counts[o].sum() for o in octs])
    T = int(np.ceil(oct_sums.max() / 128))
    while (n_grps * T) % 8 != 0:  # whole 1024-node chunks
        T += 1
    grp_nodes = T * 128
    n_pad = n_grps * grp_nodes

    starts = np.zeros(G + 1, dtype=np.int64)
    np.cumsum(counts, out=starts[1:])

    # output row (o*GRP + jj) holds graph octs[o][jj]
    gmap = np.array([g for o in octs for g in o], dtype=np.int64)

    x_bf = x.astype(BF16)
    x_f8 = x.astype(F8)  # quantized straight from fp32 (not via bf16)
    xam = np.zeros((N_CORES, n_pad, ROW), dtype=BF16)
    x8p = np.zeros((N_CORES, n_pad, H), dtype=F8)
    for c in range(N_CORES):
        xam[c, :, BLOC] = BF16(-1.0)  # padding nodes match no oct slot
    for c in range(N_CORES):
        for gl in range(n_grps):
            o = c * n_grps + gl
            base = gl * grp_nodes
            pos = base
            for jj, g in enumerate(octs[o]):
                s, e = int(starts[g]), int(starts[g + 1])
                cnt = e - s
                xam[c, pos : pos + cnt, 0:H] = x_bf[s:e]
                xam[c, pos : pos + cnt, H] = BF16(1.0)
                xam[c, pos : pos + cnt, BLOC] = BF16(jj)
                x8p[c, pos : pos + cnt] = x_f8[s:e]
                pos += cnt

    n_chunks = n_pad // 1024
    # chunk-major contiguous layouts: one big read per partition per chunk
    xam2 = np.ascontiguousarray(
        xam.reshape(N_CORES, n_chunks, 8, 128, ROW)
        .transpose(0, 1, 3, 2, 4)
        .reshape(N_CORES, n_chunks, 128, 8 * ROW)
    )
    xt8 = np.ascontiguousarray(
        x8p.transpose(0, 2, 1)  # [cores, H, n_pad]
        .reshape(N_CORES, 2, 128, n_chunks, 1024)
        .transpose(0, 3, 2, 1, 4)
        .reshape(N_CORES, n_chunks, 128, 2048)
    )
    # merged byte stream: [xam bf16 bytes | x^T fp8 bytes]
    u = np.concatenate(
        [xam2.view(np.uint8), xt8.view(np.uint8)], axis=-1
    )  # [cores, n_chunks, 128, UB]

    # consts blob
    W1hi = W1.astype(F8)
    W1lo = (W1 - W1hi.astype(np.float32)).astype(F8)
    cb = np.zeros((128, C_END), dtype=np.uint8)
    for dst, Wq in ((C_HI, W1hi), (C_LO, W1lo)):
        w1h = np.zeros((128, 512), dtype=F8)
        w1h[:, 0:256] = Wq[0:128, :]
        w1h[:, 256:512] = Wq[128:256, :]
        cb[:, dst : dst + 512] = w1h.view(np.uint8)
    w2h = np.zeros((128, 2 * HEADS), dtype=BF16)
    for ko in range(2):
        w2h[:, ko * HEADS : (ko + 1) * HEADS] = W2[
            ko * 128 : (ko + 1) * 128, :
        ].astype(BF16)
    cb[:, C_W2:C_B1] = w2h.view(np.uint8)
    b1h = np.stack([b1[0:128], b1[128:256]], axis=1).astype(np.float32)  # [128, 2]
    cb[:, C_B1:C_SHS] = b1h.view(np.uint8)
    # head-mean matrix: rows p=(o%4)*32+jj*4+h -> graph column p//4, value 1/4
    shsh = np.zeros((128, 32), dtype=BF16)
    shsh[np.arange(128), np.arange(128) // HEADS] = BF16(0.25)
    cb[:, C_SHS:C_IOT] = shsh.view(np.uint8)
    # iota over oct slots, one value per selector column, bcast to all partitions
    ioth = np.broadcast_to(
        (np.arange(SELW) // HEADS).astype(BF16)[None, :], (128, SELW)
    ).copy()
    cb[:, C_IOT:C_END] = ioth.view(np.uint8)

    return T, n_grps, u, cb, gmap


def kernel(x, batch, W1, b1, W2, num_graphs):
    global LAST_RESULT
    from concourse.bass_utils import run_bass_kernel_spmd

    x = np.asarray(x, dtype=np.float32)
    batch = np.asarray(batch).astype(np.int64)
    W1 = np.asarray(W1, dtype=np.float32)
    b1 = np.asarray(b1, dtype=np.float32)
    W2 = np.asarray(W2, dtype=np.float32)
    G = int(num_graphs)

    T, n_grps, u, cb, gmap = _host_prep(x, batch, W1, b1, W2, G)

    key = (T, n_grps)
    if key not in _NC_CACHE:
        _NC_CACHE[key] = _build_nc(T, n_grps)
    nc = _NC_CACHE[key]

    in_maps = [{"u": u[c], "c": cb} for c in range(N_CORES)]

    res = run_bass_kernel_spmd(nc, in_maps, core_ids=list(range(N_CORES)))
    LAST_RESULT = res
    raw = np.concatenate([res.results[c]["out"] for c in range(N_CORES)], axis=0)
    out = np.empty_like(raw)
    out[gmap] = raw  # undo the LPT graph permutation
    return out



# revision 38
# speedup vs baseline: 1.0053x; 1.0053x over previous
"""AttentionPooling Trainium2 kernel (8 NeuronCores, SPMD).

Reference computation:
    scores = tanh(x @ W1 + b1) @ W2          # [N, 4]
    w      = segment_softmax(scores, batch)  # per-graph softmax over nodes
    out[g] = mean_h( sum_{n in g} w[n,h] * x[n] )   # [G, 256]

Sharding: 64 graphs per core (512 graphs / 8 cores), LPT-bin-packed into
octs of 8 graphs so the largest oct is minimal; each oct's nodes are padded
to a fixed number of 128-node tiles (T) so every core runs the identical
instruction stream.  Weights are replicated; per-graph outputs are disjoint,
so the host concatenates the 8 core outputs and undoes the LPT permutation.

On-core algorithm (single pass over x, one 1024-node chunk at a time):
  - two DMA streams per chunk from one DRAM blob: the packed node-major row
    [x bf16 | 1 | bloc] on the SP-HWDGE path (two half-row transfers for a
    finer wire interleave), and x^T in e4m3 on the SWDGE path.  The x^T
    ring is deep and the xam ring is shallow, so the score-critical fp8
    bytes win the early wire slots.  A PE warm-up burst and a dummy
    activation at t~0 pay the p-state ramp and the ~1.3us activation-table
    load before the first real data arrives.
  - z^T = W1^T @ x^T on TensorE as fp8 DoubleRow matmuls (K=256 folded into
    one pass); W1 is sent as an e4m3 hi+lo pair so its quantization error
    cancels (W1hi + W1lo ~ bf16-accurate), x^T is single e4m3
  - tanh(+per-partition bias) on ScalarE in two 1024-wide calls per chunk
    (one per h_out half) -> t^T bf16; ScalarE is the pacing engine, so exp
    is batched over three chunks (SB=3) to cut per-call overhead
  - s (node-major, 4 heads): per (tile j, ko) one M=128 matmul with the
    t^T slice stationary and W2 moving (out free size 4); the ko pair is
    emitted adjacently so each PSUM region's start/stop closes immediately.
  - oct one-hot mask on DVE from the bloc column via a broadcast-AP
    iota-compare; sel[128, 32/tile] = e (broadcast over 8 oct slots) * mask
  - pooled[(oct%4)*32 + slot*4 + head, 0:256] += sel^T @ [x | 1] in bf16,
    accumulated across the whole shard in 2 persistent PSUM banks;
    column 256 (the ones column) gives the softmax denominator
  - epilogue per bank: clamp+divide by denominator, average heads with a
    constant matmul back into the same PSUM bank, copy out.  Bank A covers
    octs 0-3 and finishes mid-kernel, so its epilogue compute is emitted
    right after its last accumulation and hides under the main loop; the
    tiny out-DMAs run at the end (a mid-loop SP dma_start would stall the
    in-order SP queue, which prefetches chunks ahead).
"""

import contextlib as _ctxlib
import numpy as np
import ml_dtypes


def _nullctx():
    return _ctxlib.nullcontext()

BF16 = ml_dtypes.bfloat16
F8 = ml_dtypes.float8_e4m3  # IEEE e4m3 (max 240) == TRN FP8_EXP4

N_CORES = 8
H = 256
HEADS = 4
GRP = 8  # graphs per oct group
SELW = GRP * HEADS  # 32 selector columns per node
ROW = H + 2  # packed row: x(256) | ones(1) | bloc(1)
BLOC = H + 1  # bloc column index

XAMB = 8 * ROW * 2  # xam bytes per partition per chunk (4128)
XTB = 2 * 1024  # x^T fp8 bytes per partition per chunk (2048)
UB = XAMB + XTB  # merged stream bytes per partition per chunk (6176)
# consts blob byte offsets: W1hi | W1lo | W2 | b1 | shs | iota
C_HI, C_LO, C_W2, C_B1, C_SHS, C_IOT, C_END = 0, 512, 1024, 1040, 1048, 1112, 1176

_NC_CACHE = {}
LAST_RESULT = None

# pipeline tuning (timeline-sim validated: 66625 ns vs 66976 baseline)
SB_CHUNKS = 3  # chunks per exp superblock
XAM_BUFS = 5  # node-major ring depth (chunks)
XT8_BUFS = 6  # x^T fp8 ring depth (chunks)


def _build_nc(T: int, n_grps: int, repeats: int = 1):
    """Build the SPMD Bass program. T = 128-node tiles per oct group."""
    import concourse.bacc as bacc
    import concourse.mybir as mybir
    from concourse.tile import TileContext

    fp32 = mybir.dt.float32
    bf16 = mybir.dt.bfloat16
    f8 = mybir.dt.float8e4
    u8 = mybir.dt.uint8
    AF = mybir.ActivationFunctionType
    DR = mybir.MatmulPerfMode.DoubleRow

    n_tiles = n_grps * T
    assert n_tiles % 8 == 0
    n_chunks = n_tiles // 8  # 1024-node chunks
    assert n_grps == 8, "psum layout assumes 8 octs (64 graphs) per core"
    ch_a_done = (4 * T - 1) // 8  # chunk in which poolA's last oct closes

    nc = bacc.Bacc(trn_type="TRN2")

    u = nc.dram_tensor("u", [n_chunks, 128, UB], u8, kind="ExternalInput")
    cdram = nc.dram_tensor("c", [128, C_END], u8, kind="ExternalInput")
    out = nc.dram_tensor("out", [64, H], fp32, kind="ExternalOutput")

    with TileContext(nc, pool_alloc_mode="queue") as tc:
        with (
            tc.tile_pool(name="consts", bufs=1) as cpool,
            tc.tile_pool(name="acc", bufs=1, space="PSUM") as acc_pool,
        ):
            cub = cpool.tile([128, C_END], u8)
            nc.sync.dma_start(cub[:], cdram.ap())
            # typed views into the consts blob
            w1hi = cub[:, C_HI:C_LO].bitcast(f8).rearrange("p (k m) -> p k m", k=2)
            w1lo = cub[:, C_LO:C_W2].bitcast(f8).rearrange("p (k m) -> p k m", k=2)
            w2v = cub[:, C_W2:C_B1].bitcast(bf16)  # [128, 8]
            b1v = cub[:, C_B1:C_SHS].bitcast(fp32)  # [128, 2]
            shsv = cub[:, C_SHS:C_IOT].bitcast(bf16)  # [128, 32]
            iotv = cub[:, C_IOT:C_END].bitcast(bf16)  # [128, 32]

            # persistent accumulators: rows = (oct%4)*32 + jj*4 + h, col 256 = seg_e
            poolA = acc_pool.tile([128, H + 1], fp32)
            poolB = acc_pool.tile([128, H + 1], fp32)

            def epilogue(ps, idx, wp, on_act):
                """Normalize by seg_e, head-mean back into ps[0:32].

                Returns the SBUF result tile; the caller issues the out-DMA.
                """
                seg = wp.tile([128, 1], fp32, name=f"seg{idx}", tag=f"seg{idx}")
                nc.vector.tensor_scalar(
                    seg[:], ps[:, H : H + 1], 1e-30, None, mybir.AluOpType.max
                )
                rec = wp.tile([128, 1], fp32, name=f"rec{idx}", tag=f"rec{idx}")
                nc.vector.reciprocal(rec[:], seg[:])
                norm = wp.tile([128, H], bf16, name=f"norm{idx}", tag=f"norm{idx}")
                nc.vector.tensor_scalar(
                    norm[:], ps[:, 0:H], rec[:], None, mybir.AluOpType.mult
                )
                # head-mean matmul reuses the accumulator's own PSUM bank
                nc.tensor.matmul(ps[0:32, 0:H], shsv, norm[:], start=True, stop=True)
                osb = wp.tile([32, H], fp32, name=f"osb{idx}", tag=f"osb{idx}")
                if on_act:
                    nc.scalar.copy(osb[:], ps[0:32, 0:H])
                else:
                    nc.vector.tensor_scalar(
                        osb[:], ps[0:32, 0:H], 0.0, None, mybir.AluOpType.add
                    )
                return osb

            # PE warm-up: zero matmuls keep the PE busy from t~0 so the
            # p-state ramp (2x slower below 3us) is over before chunk 0, and
            # a dummy activation on the same tile pulls the ~1.3us
            # LoadActFuncSet (tanh/exp table set) off the ramp
            wrm = cpool.tile([128, 257], bf16)
            dummy = cpool.tile([1, 8], bf16)
            with tc.high_priority():
                nc.vector.memset(wrm[:], 0.0)
                nc.scalar.activation(dummy[:], wrm[0:1, 0:8], AF.Tanh)
                for _w in range(14):
                    nc.tensor.matmul(
                        poolA[:], wrm[:, 0:128], wrm[:], start=True, stop=True
                    )

            with (
                # Two independent DMA rings.  The deep x^T ring lets the
                # fp8 score stream run ahead on the wire (it gates the
                # ACT-critical tanh chain); the shallow xam ring is
                # throttled by its pool-matmul readers, so the bulky
                # node-major bytes naturally yield the early wire slots.
                tc.tile_pool(name="xt8", bufs=XT8_BUFS) as xpool,
                tc.tile_pool(name="xam", bufs=XAM_BUFS) as apool,
                tc.tile_pool(name="work", bufs=5) as wpool,
                tc.tile_pool(name="zt", bufs=1, space="PSUM") as zpool,
                tc.tile_pool(name="sps", bufs=2, space="PSUM") as spool,
            ):
              SB = SB_CHUNKS  # chunks per exp superblock

              for _rep in range(repeats):
                s_ps = None
                ub_sb = [None] * SB
                osb_a = None
                for ch in range(n_chunks):
                    xt = xpool.tile([128, XTB], u8, name="xt")
                    nc.gpsimd.dma_start(xt[:], u.ap()[ch, :, XAMB:UB])
                    ub = apool.tile([128, XAMB], u8, name="am")
                    ub_sb[ch % SB] = ub
                    # two half-row DMAs: finer wire interleave, so the fp8
                    # x^T pieces get slots sooner during the ramp
                    nc.sync.dma_start(
                        ub[:, 0 : XAMB // 2], u.ap()[ch, :, 0 : XAMB // 2]
                    )
                    nc.sync.dma_start(
                        ub[:, XAMB // 2 : XAMB], u.ap()[ch, :, XAMB // 2 : XAMB]
                    )
                    xamv = ub[:].bitcast(bf16)  # [128, 2064]
                    xt8v = xt[:].bitcast(f8).rearrange(
                        "p (k n) -> p k n", k=2
                    )  # [128, 2, 1024]

                    # z^T = (W1hi + W1lo)^T @ x^T, fp8 DoubleRow (K=256/pass)
                    tt = wpool.tile([128, 2048], bf16, name="tt", tag="tt")
                    for ko in range(2):
                        zt = zpool.tile(
                            [128, 1024], fp32, name=f"zt{ko}", tag=f"zt{ko}"
                        )
                        for s2 in range(2):
                            # high priority: these gate the ACT-critical tanh
                            # chain and must not queue behind the pool backlog
                            with tc.high_priority():
                                nc.tensor.matmul(
                                    zt[:, s2 * 512 : (s2 + 1) * 512],
                                    w1hi[:, :, ko * 128 : ko * 128 + 128],
                                    xt8v[:, :, s2 * 512 : (s2 + 1) * 512],
                                    start=True,
                                    stop=False,
                                    perf_mode=DR,
                                )
                                nc.tensor.matmul(
                                    zt[:, s2 * 512 : (s2 + 1) * 512],
                                    w1lo[:, :, ko * 128 : ko * 128 + 128],
                                    xt8v[:, :, s2 * 512 : (s2 + 1) * 512],
                                    start=False,
                                    stop=True,
                                    perf_mode=DR,
                                )
                        # tanh(+bias): per-partition bias, one 1024-wide call
                        nc.scalar.activation(
                            tt[:, ko * 1024 : (ko + 1) * 1024],
                            zt[:],
                            AF.Tanh,
                            bias=b1v[:, ko : ko + 1],
                        )

                    # s (node-major, 4 heads): per (tile j, ko) one M=128
                    # matmul with the tt slice stationary (cost-free) and W2
                    # moving (out free size 4).  ko pairs adjacent so each
                    # PSUM region's start/stop closes immediately.
                    cpar = ch % SB
                    if cpar == 0:
                        s_ps = spool.tile(
                            [128, SB * SELW], fp32, name="s_ps", tag="s_ps"
                        )
                    for j in range(8):
                        for ko in range(2):
                            with tc.high_priority():
                                nc.tensor.matmul(
                                    s_ps[
                                        :,
                                        cpar * SELW + j * HEADS : cpar * SELW
                                        + (j + 1) * HEADS,
                                    ],
                                    tt[
                                        :,
                                        ko * 1024 + j * 128 : ko * 1024
                                        + j * 128
                                        + 128,
                                    ],
                                    w2v[:, ko * HEADS : (ko + 1) * HEADS],
                                    start=(ko == 0),
                                    stop=(ko == 1),
                                )
                    # e = exp(s), once per superblock (SB chunks)
                    last_in_sb = cpar == SB - 1 or ch == n_chunks - 1
                    if last_in_sb:
                        ew = (cpar + 1) * SELW
                        e_sb = wpool.tile([128, SB * SELW], bf16, name="e_sb")
                        # nudged later in the ACT queue: the s_ps->exp sem
                        # latency then hides under the next chunk's tanh.
                        # Near the end run it at natural priority instead, so
                        # the final superblocks' pool matmuls don't bunch up
                        # after the last tanh.
                        if ch >= n_chunks - 8:
                            nc.scalar.activation(e_sb[:, 0:ew], s_ps[:, 0:ew], AF.Exp)
                        else:
                            with tc.high_priority(offset=-45):
                                nc.scalar.activation(
                                    e_sb[:, 0:ew], s_ps[:, 0:ew], AF.Exp
                                )
                        for cc in range(cpar + 1):
                            cx = ch - cpar + cc
                            _chunk_tail(
                                nc, mybir, tc, wpool, ub_sb[cc], e_sb, cc, cx,
                                T, poolA, poolB, iotv,
                                last=(cx >= n_chunks - 3),
                            )
                            if cx == ch_a_done and _rep == repeats - 1:
                                # deprioritized: fills engine gaps, must not
                                # displace the steady-state pipeline
                                with tc.high_priority(offset=-100000):
                                    osb_a = epilogue(poolA, 0, wpool, on_act=False)

                # bank B epilogue (and bank A's if it didn't fire mid-loop)
                if _rep == repeats - 1:
                    if osb_a is None:
                        osb_a = epilogue(poolA, 0, wpool, on_act=False)
                    nc.sync.dma_start(out.ap()[0:32, :], osb_a[:])
                    with tc.high_priority():
                        osb_b = epilogue(poolB, 1, wpool, on_act=True)
                    nc.sync.dma_start(out.ap()[32:64, :], osb_b[:])

    nc.finalize()
    return nc


def _chunk_tail(
    nc, mybir, tc, wpool, ub, e_sb, cc, cx, T, poolA, poolB, iotv, last=False
):
    """Mask, selector, and pool matmuls for chunk cx (parity slot cc).

    For the final chunk the selector multiply is split into j-halves so the
    last pool matmuls (the end-of-program critical chain) start earlier.
    """
    bf16 = mybir.dt.bfloat16
    xamv = ub[:].bitcast(bf16)
    # oct one-hot masks: (bloc == iota), one broadcast op.  High priority:
    # no exp dependency, so it can run as soon as the xam bytes land instead
    # of queueing behind the superblock tail.
    mk = wpool.tile([128, 8 * SELW], bf16, name="mk")
    bloc_b = (
        xamv.rearrange("p (j c) -> p j c", j=8)[:, :, BLOC : BLOC + 1]
        .broadcast_to((128, 8, SELW))
    )
    iot_b = iotv.rearrange("p (o c) -> p o c", o=1).broadcast_to((128, 8, SELW))
    ctx = tc.high_priority() if last else _nullctx()
    with ctx:
        nc.vector.tensor_tensor(
            mk[:].rearrange("p (j c) -> p j c", j=8),
            bloc_b,
            iot_b,
            mybir.AluOpType.is_equal,
        )
    # selector = e * mask (e broadcast over the 8 oct slots)
    sel = wpool.tile([128, 8 * SELW], bf16, name="sel")
    for h0, h1 in ([(0, 4), (4, 8)] if last else [(0, 8)]):
        e_b = (
            e_sb[:, cc * SELW + h0 * HEADS : cc * SELW + h1 * HEADS]
            .rearrange("p (j o h) -> p j o h", j=h1 - h0, o=1)
            .broadcast_to((128, h1 - h0, GRP, HEADS))
        )
        nc.vector.tensor_tensor(
            sel[:, h0 * SELW : h1 * SELW].rearrange(
                "p (j o h) -> p j o h", j=h1 - h0, o=GRP
            ),
            e_b,
            mk[:, h0 * SELW : h1 * SELW].rearrange(
                "p (j o h) -> p j o h", j=h1 - h0, o=GRP
            ),
            mybir.AluOpType.mult,
        )
    # pooled[(o%4)*32 : +32, :] += sel_j^T @ [x_j | 1]
    for j in range(8):
        t_glob = cx * 8 + j
        o = t_glob // T
        tau = t_glob % T
        ps = poolA if (o % 8) < 4 else poolB
        r0 = (o % 4) * 32
        nc.tensor.matmul(
            ps[r0 : r0 + 32, :],
            sel[:, j * SELW : (j + 1) * SELW],
            xamv[:, j * ROW : j * ROW + H + 1],
            start=(tau == 0),
            stop=(tau == T - 1),
            tile_position=(0, r0),
        )


def _lpt_octs(counts, n_octs):
    """LPT-pack graphs into octs of GRP graphs, minimizing the max oct size."""
    import heapq

    order = np.argsort(-counts)
    heap = [(0, i, []) for i in range(n_octs)]
    heapq.heapify(heap)
    for g in order:
        popped = []
        while True:
            sz, i, lst = heapq.heappop(heap)
            if len(lst) < GRP:
                break
            popped.append((sz, i, lst))
        heapq.heappush(heap, (sz + int(counts[g]), i, lst + [int(g)]))
        for p in popped:
            heapq.heappush(heap, p)
    octs = [None] * n_octs
    for sz, i, lst in heap:
        octs[i] = lst
    return octs


def _host_prep(x, batch, W1, b1, W2, G):
    """Shard + pad inputs; build all per-core DRAM arrays."""
    gpc = G // N_CORES  # graphs per core
    n_grps = gpc // GRP  # oct groups per core
    counts = np.bincount(batch, minlength=G)
    octs = _lpt_octs(counts, G // GRP)  # balanced graph -> oct assignment
    oct_sums = np.array([counts[o].sum() for o in octs])
    T = int(np.ceil(oct_sums.max() / 128))
    while (n_grps * T) % 8 != 0:  # whole 1024-node chunks
        T += 1
    grp_nodes = T * 128
    n_pad = n_grps * grp_nodes

    starts = np.zeros(G + 1, dtype=np.int64)
    np.cumsum(counts, out=starts[1:])

    # output row (o*GRP + jj) holds graph octs[o][jj]
    gmap = np.array([g for o in octs for g in o], dtype=np.int64)

    x_bf = x.astype(BF16)
    x_f8 = x.astype(F8)  # quantized straight from fp32 (not via bf16)
    xam = np.zeros((N_CORES, n_pad, ROW), dtype=BF16)
    x8p = np.zeros((N_CORES, n_pad, H), dtype=F8)
    for c in range(N_CORES):
        xam[c, :, BLOC] = BF16(-1.0)  # padding nodes match no oct slot
    for c in range(N_CORES):
        for gl in range(n_grps):
            o = c * n_grps + gl
            base = gl * grp_nodes
            pos = base
            for jj, g in enumerate(octs[o]):
                s, e = int(starts[g]), int(starts[g + 1])
                cnt = e - s
                xam[c, pos : pos + cnt, 0:H] = x_bf[s:e]
                xam[c, pos : pos + cnt, H] = BF16(1.0)
                xam[c, pos : pos + cnt, BLOC] = BF16(jj)
                x8p[c, pos : pos + cnt] = x_f8[s:e]
                pos += cnt

    n_chunks = n_pad // 1024
    # chunk-major contiguous layouts: one big read per partition per chunk
    xam2 = np.ascontiguousarray(
        xam.reshape(N_CORES, n_chunks, 8, 128, ROW)
        .transpose(0, 1, 3, 2, 4)
        .reshape(N_CORES, n_chunks, 128, 8 * ROW)
    )
    xt8 = np.ascontiguousarray(
        x8p.transpose(0, 2, 1)  # [cores, H, n_pad]
        .reshape(N_CORES, 2, 128, n_chunks, 1024)
        .transpose(0, 3, 2, 1, 4)
        .reshape(N_CORES, n_chunks, 128, 2048)
    )
    # merged byte stream: [xam bf16 bytes | x^T fp8 bytes]
    u = np.concatenate(
        [xam2.view(np.uint8), xt8.view(np.uint8)], axis=-1
    )  # [cores, n_chunks, 128, UB]

    # consts blob
    W1hi = W1.astype(F8)
    W1lo = (W1 - W1hi.astype(np.float32)).astype(F8)
    cb = np.zeros((128, C_END), dtype=np.uint8)
    for dst, Wq in ((C_HI, W1hi), (C_LO, W1lo)):
        w1h = np.zeros((128, 512), dtype=F8)
        w1h[:, 0:256] = Wq[0:128, :]
        w1h[:, 256:512] = Wq[128:256, :]
        cb[:, dst : dst + 512] = w1h.view(np.uint8)
    w2h = np.zeros((128, 2 * HEADS), dtype=BF16)
    for ko in range(2):
        w2h[:, ko * HEADS : (ko + 1) * HEADS] = W2[
            ko * 128 : (ko + 1) * 128, :
        ].astype(BF16)
    cb[:, C_W2:C_B1] = w2h.view(np.uint8)
    b1h = np.stack([b1[0:128], b1[128:256]], axis=1).astype(np.float32)  # [128, 2]
    cb[:, C_B1:C_SHS] = b1h.view(np.uint8)
    # head-mean matrix: rows p=(o%4)*32+jj*4+h -> graph column p//4, value 1/4
    shsh = np.zeros((128, 32), dtype=BF16)
    shsh[np.arange(128), np.arange(128) // HEADS] = BF16(0.25)
    cb[:, C_SHS:C_IOT] = shsh.view(np.uint8)
    # iota over oct slots, one value per selector column, bcast to all partitions
    ioth = np.broadcast_to(
        (np.arange(SELW) // HEADS).astype(BF16)[None, :], (128, SELW)
    ).copy()
    cb[:, C_IOT:C_END] = ioth.view(np.uint8)

    return T, n_grps, u, cb, gmap


def kernel(x, batch, W1, b1, W2, num_graphs):
    global LAST_RESULT
    from concourse.bass_utils import run_bass_kernel_spmd

    x = np.asarray(x, dtype=np.float32)
    batch = np.asarray(batch).astype(np.int64)
    W1 = np.asarray(W1, dtype=np.float32)
    b1 = np.asarray(b1, dtype=np.float32)
    W2 = np.asarray(W2, dtype=np.float32)
    G = int(num_graphs)

    T, n_grps, u, cb, gmap = _host_prep(x, batch, W1, b1, W2, G)

    key = (T, n_grps)
    if key not in _NC_CACHE:
        _NC_CACHE[key] = _build_nc(T, n_grps)
    nc = _NC_CACHE[key]

    in_maps = [{"u": u[c], "c": cb} for c in range(N_CORES)]

    res = run_bass_kernel_spmd(nc, in_maps, core_ids=list(range(N_CORES)))
    LAST_RESULT = res
    raw = np.concatenate([res.results[c]["out"] for c in range(N_CORES)], axis=0)
    out = np.empty_like(raw)
    out[gmap] = raw  # undo the LPT graph permutation
    return out
